# revision 1
# baseline (speedup 1.0000x reference)
"""Trainium2 Bass kernel v2: 2-layer LLaMA-style transformer, 8-way tensor
parallel, sequence-chunked pipeline.

Per core:
- Activations feature-major [feat, seq] in SBUF; weights host-packed into
  partition-major slabs so every weight DMA is one long-line transfer.
- Sequence processed in CH chunks; each half-layer's AllReduce issued per
  chunk so collectives overlap neighboring chunks' compute.
- RMSNorm 1/rms folded into PSUM evictions (and into rope tables for q/k);
  ln weights folded into weights on host.
- Residual folded into the AllReduce: AR input = partial_out + 0.125*h via
  an extra identity matmul into the same PSUM accumulation, so
  h_new = AllReduce(.) with no separate residual pass.
"""

from contextlib import ExitStack

import numpy as np

import concourse.bass as bass
import concourse.bacc as bacc
import concourse.tile as tile
from concourse import mybir
from concourse.bass_utils import run_bass_kernel_spmd

F32 = mybir.dt.float32
H16 = mybir.dt.float16
AF = mybir.ActivationFunctionType

B, S = 1, 1024
V, H, NH, HD, F, L, O = 32000, 4096, 32, 128, 11008, 2, 512
NCORES = 8
NH_C = NH // NCORES      # 4 heads/core
DQ = NH_C * HD           # 512
F_C = F // NCORES        # 1376
F_CP = 1408              # 11*128
O_C = O // NCORES        # 64
HT = H // 128            # 32
FT = F_CP // 128         # 11
ROPE_THETA = 10000.0
EPS = 1e-6
NEG = -30000.0

CH = 4                   # sequence chunks
CS = S // CH             # 256
CT = CS // 128           # s-tiles per chunk

QKM = 2 * DQ // 128      # 8 qk slabs
GUM = 2 * FT             # 22 gu slabs (g0,u0,g1,u1,...)
WVG = 4                  # wv k-tiles per DMA group
WOG = 4                  # wo slabs per DMA group
WDG = 2                  # wd slabs per DMA group


def _ap3(ap2, p, d1, d2, s1):
    """3D view [(p,128), (d1, stride s1), (d2, contiguous)] of a 2D AP."""
    return bass.AP(tensor=ap2.tensor, offset=ap2.offset,
                   ap=[[ap2.ap[0][0], p], [s1, d1], [ap2.ap[1][0], d2]])


def build_nc():
    nc = bacc.Bacc("TRN2", target_bir_lowering=False, debug=False,
                   num_devices=NCORES)

    # ---- kernel I/O ----
    h0T = nc.dram_tensor("h0T", [H, S], H16, kind="ExternalInput")
    cos2_d = nc.dram_tensor("cos2", [128, S], H16, kind="ExternalInput")
    sin2_d = nc.dram_tensor("sin2", [128, S], H16, kind="ExternalInput")
    maskT_d = nc.dram_tensor("maskT", [128, 128], H16, kind="ExternalInput")
    id8_d = nc.dram_tensor("id8", [128, 128], H16, kind="ExternalInput")
    wqk = [nc.dram_tensor(f"wqk{l}", [QKM * 128, HT * 128], H16,
                          kind="ExternalInput") for l in range(L)]
    wv = [nc.dram_tensor(f"wv{l}", [128, HT * DQ], H16,
                         kind="ExternalInput") for l in range(L)]
    wo = [nc.dram_tensor(f"wo{l}", [HT * 128, DQ], H16,
                         kind="ExternalInput") for l in range(L)]
    wgu = [nc.dram_tensor(f"wgu{l}", [GUM * 128, HT * 128], H16,
                          kind="ExternalInput") for l in range(L)]
    wd = [nc.dram_tensor(f"wd{l}", [HT * 128, FT * 128], H16,
                         kind="ExternalInput") for l in range(L)]
    wout_d = nc.dram_tensor("wout", [128, HT * O_C], H16,
                            kind="ExternalInput")
    bout_d = nc.dram_tensor("bout", [O_C, 1], F32, kind="ExternalInput")
    outT = nc.dram_tensor("outT", [O_C, S], F32, kind="ExternalOutput")

    with tile.TileContext(nc) as tc, ExitStack() as ctx:
        ec = ctx.enter_context
        dpool = ec(tc.tile_pool(name="drams", bufs=1, space="DRAM"))
        cpool = ec(tc.tile_pool(name="consts", bufs=1))
        apool = ec(tc.tile_pool(name="acts", bufs=1))      # qkr/vnat/attnT
        hpool = ec(tc.tile_pool(name="hsb", bufs=2))       # h chunks
        stpool = ec(tc.tile_pool(name="stp", bufs=4))      # binv/cosI/...
        sTpool = ec(tc.tile_pool(name="sTs", bufs=2))      # swiglu out
        wpool = ec(tc.tile_pool(name="wslab", bufs=2))     # wqk/wgu/wout
        wvpool = ec(tc.tile_pool(name="wvg", bufs=2))      # wv groups
        wopool = ec(tc.tile_pool(name="wog", bufs=2))      # wo groups
        wdpool = ec(tc.tile_pool(name="wdg", bufs=2))      # wd groups
        spool = ec(tc.tile_pool(name="small", bufs=4))
        aspool = ec(tc.tile_pool(name="asb", bufs=1))      # attn exp tiles
        evpool = ec(tc.tile_pool(name="evict", bufs=2))
        arpool = ec(tc.tile_pool(name="arb", bufs=1))      # AR-input stage
        pspool = ec(tc.tile_pool(name="ps", bufs=2, space="PSUM"))
        ps1pool = ec(tc.tile_pool(name="ps1", bufs=1, space="PSUM"))

        # ---- constants ----
        cos2 = cpool.tile([128, S], H16, name="cos2sb")
        nc.sync.dma_start(out=cos2, in_=cos2_d.ap())
        sin2 = cpool.tile([128, S], H16, name="sin2sb")
        nc.sync.dma_start(out=sin2, in_=sin2_d.ap())
        maskT = cpool.tile([128, 128], H16, name="maskTsb")
        nc.sync.dma_start(out=maskT, in_=maskT_d.ap())
        id8 = cpool.tile([128, 128], H16, name="id8sb")
        nc.sync.dma_start(out=id8, in_=id8_d.ap())
        maskF = cpool.tile([128, 128], H16, name="maskFsb")
        nc.vector.memset(maskF, NEG)
        ones128 = cpool.tile([128, 1], H16, name="ones128")
        nc.vector.memset(ones128, 1.0)
        ones1 = cpool.tile([1, 128], F32, name="ones1")
        nc.vector.memset(ones1, 1.0)
        eps1 = cpool.tile([1, 1], F32, name="eps1")
        nc.vector.memset(eps1, EPS)

        # ---- persistent per-layer activations ----
        qkr = apool.tile([128, 2 * NH_C, S], H16, name="qkr")     # 16KB/part
        vnat = apool.tile([128, S // 128, DQ], H16, name="vnat")  # 8KB/part
        attnT = apool.tile([128, NH_C, S], H16, name="attnT")     # 8KB/part

        # ---- DRAM scratch ----
        invd = dpool.tile([1, S], F32, name="invd")
        ari = [[[dpool.tile([H, CS], H16, name=f"ari{l}_{hf}_{c}")
                 for c in range(CH)] for hf in range(2)] for l in range(L)]
        aro = [[[dpool.tile([H, CS], H16, name=f"aro{l}_{hf}_{c}",
                            addr_space="Shared")
                 for c in range(CH)] for hf in range(2)] for l in range(L)]

        def h_src_ap(l, hf, c):
            if l == 0 and hf == 0:
                return h0T.ap()[:, c * CS:(c + 1) * CS]
            if hf == 0:
                return aro[l - 1][1][c][:, :]
            return aro[l][0][c][:, :]

        def load_h_chunk(l, hf, c):
            """One DMA: [H, CS] dram slice -> SBUF [128, HT*CS] (kt-major)."""
            t = hpool.tile([128, HT * CS], H16, name="hsb")
            src = h_src_ap(l, hf, c)
            nc.sync.dma_start(
                out=_ap3(t[:, :], 128, HT, CS, CS),
                in_=_ap3(src, 128, HT, CS, src.ap[0][0] * 128))
            return t

        def store_ar_chunk(arbuf, dst):
            """One DMA: SBUF [128, HT*CS] -> [H, CS] dram."""
            nc.sync.dma_start(
                out=_ap3(dst, 128, HT, CS, dst.ap[0][0] * 128),
                in_=_ap3(arbuf[:, :], 128, HT, CS, CS))

        def stats(hsb, c, want_invcol=False, want_rope=False):
            """ssq -> 1/rms; binv16 [128,CS] broadcast; optional invcol
            [128,CT] and rope tables pre-scaled by inv."""
            pss = ps1pool.tile([1, 512], F32, name="pstat")
            for kt in range(HT):
                sl = hsb[:, kt * CS:(kt + 1) * CS]
                sq = spool.tile([128, CS], H16, name="sqt")
                nc.vector.tensor_mul(out=sq, in0=sl, in1=sl)
                nc.tensor.matmul(pss[:, :CS], ones128, sq,
                                 start=(kt == 0), stop=(kt == HT - 1))
            srow = spool.tile([1, CS], F32, name="srow")
            nc.scalar.activation(out=srow, in_=pss[:, :CS], func=AF.Sqrt,
                                 bias=eps1, scale=1.0 / H)
            inv = spool.tile([1, CS], F32, name="invrow")
            nc.vector.reciprocal(out=inv, in_=srow)
            pbc = ps1pool.tile([128, 512], F32, name="pbcast")
            nc.tensor.matmul(pbc[:, :CS], ones1, inv, start=True, stop=True)
            binv = stpool.tile([128, CS], H16, name="binv")
            nc.vector.tensor_copy(out=binv, in_=pbc[:, :CS])
            out = {"binv": binv}
            if want_invcol:
                dsl = invd[:, c * CS:(c + 1) * CS]
                nc.sync.dma_start(out=dsl, in_=inv)
                invcol = stpool.tile([128, CT], F32, name="invcol")
                nc.sync.dma_start(out=invcol, in_=bass.AP(
                    tensor=dsl.tensor, offset=dsl.offset,
                    ap=[[1, 128], [128, CT]]))
                out["invcol"] = invcol
            if want_rope:
                csl = slice(c * CS, (c + 1) * CS)
                cosI = stpool.tile([128, CS], H16, name="cosI")
                nc.vector.tensor_tensor(out=cosI, in0=cos2[:, csl], in1=binv,
                                        op=mybir.AluOpType.mult)
                sinI = stpool.tile([128, CS], H16, name="sinI")
                nc.vector.tensor_tensor(out=sinI, in0=sin2[:, csl], in1=binv,
                                        op=mybir.AluOpType.mult)
                out["cosI"] = cosI
                out["sinI"] = sinI
            return out

        # =========================== layers ===========================
        for l in range(L):
            # ---------------- phase A: attention (chunk-outer) ----------
            for c in range(CH):
                hsb = load_h_chunk(l, 0, c)
                st = stats(hsb, c, want_invcol=True, want_rope=True)

                # qk projections + fused rope
                for m in range(QKM):
                    wsl = wpool.tile([128, 2 * HT * 128], H16, name="wslab")
                    nc.sync.dma_start(
                        out=wsl[:, :HT * 128],
                        in_=wqk[l].ap()[m * 128:(m + 1) * 128, :])
                    pm = pspool.tile([128, 512], F32, name="mmps")
                    for kt in range(HT):
                        nc.tensor.matmul(
                            pm[:, :CS], wsl[:, kt * 128:(kt + 1) * 128],
                            hsb[:, kt * CS:(kt + 1) * CS],
                            start=(kt == 0), stop=(kt == HT - 1))
                    # rope fused into eviction (tables carry 1/rms)
                    qk16 = evpool.tile([128, CS], H16, name="qk16")
                    nc.vector.tensor_copy(out=qk16, in_=pm[:, :CS])
                    rot = evpool.tile([128, CS], H16, name="rot")
                    nc.scalar.mul(out=rot[0:64, :], in_=qk16[64:128, :],
                                  mul=-1.0)
                    nc.scalar.copy(out=rot[64:128, :], in_=qk16[0:64, :])
                    t1 = evpool.tile([128, CS], H16, name="ropet1")
                    nc.vector.tensor_tensor(out=t1, in0=qk16, in1=st["cosI"],
                                            op=mybir.AluOpType.mult)
                    t2 = evpool.tile([128, CS], H16, name="ropet2")
                    nc.vector.tensor_tensor(out=t2, in0=rot, in1=st["sinI"],
                                            op=mybir.AluOpType.mult)
                    nc.vector.tensor_add(
                        out=qkr[:, m, c * CS:(c + 1) * CS], in0=t1, in1=t2)

                # v projection (natural layout), wv streamed in kt groups
                pvs = [pspool.tile([128, 512], F32, name="psc")
                       for _ in range(CT)]
                for g in range(HT // WVG):
                    wvt = wvpool.tile([128, WVG * DQ], H16, name="wvg")
                    nc.sync.dma_start(
                        out=wvt,
                        in_=wv[l].ap()[:, g * WVG * DQ:(g + 1) * WVG * DQ])
                    for j in range(WVG):
                        kt = g * WVG + j
                        for stt in range(CT):
                            nc.tensor.matmul(
                                pvs[stt],
                                hsb[:, kt * CS + stt * 128:
                                    kt * CS + (stt + 1) * 128],
                                wvt[:, j * DQ:(j + 1) * DQ],
                                start=(kt == 0), stop=(kt == HT - 1))
                for stt in range(CT):
                    nc.vector.tensor_scalar_mul(
                        out=vnat[:, c * CT + stt, :], in0=pvs[stt],
                        scalar1=st["invcol"][:, stt:stt + 1])

                # attention for chunk c
                nkt = CT * (c + 1)
                for hh in range(NH_C):
                    a_sb = aspool.tile([128, S // 128, CS], H16, name="asb")
                    for kt in range(nkt):
                        psc = pspool.tile([128, 512], F32, name="psc")
                        nc.tensor.matmul(
                            psc[:, :CS],
                            qkr[:, NH_C + hh, kt * 128:(kt + 1) * 128],
                            qkr[:, hh, c * CS:(c + 1) * CS],
                            start=True, stop=True)
                        dj = kt - CT * c
                        if dj >= 1:
                            nc.vector.tensor_add(
                                out=psc[:, 0:128], in0=psc[:, 0:128],
                                in1=maskF)
                        if 0 <= dj < CT:
                            nc.vector.tensor_add(
                                out=psc[:, dj * 128:(dj + 1) * 128],
                                in0=psc[:, dj * 128:(dj + 1) * 128],
                                in1=maskT)
                        nc.scalar.activation(out=a_sb[:, kt, :],
                                             in_=psc[:, :CS], func=AF.Exp)
                    po = ps1pool.tile([128, 512], F32, name="po")
                    pd = ps1pool.tile([1, 512], F32, name="pd")
                    for kt in range(nkt):
                        nc.tensor.matmul(po[:, :CS],
                                         vnat[:, kt, hh * 128:(hh + 1) * 128],
                                         a_sb[:, kt, :],
                                         start=(kt == 0), stop=(kt == nkt - 1))
                        nc.tensor.matmul(pd[:, :CS], ones128, a_sb[:, kt, :],
                                         start=(kt == 0), stop=(kt == nkt - 1))
                    den = spool.tile([1, CS], F32, name="den")
                    nc.vector.reciprocal(out=den, in_=pd[:, :CS])
                    pbd = ps1pool.tile([128, 512], F32, name="pbcast")
                    nc.tensor.matmul(pbd[:, :CS], ones1, den,
                                     start=True, stop=True)
                    bden = evpool.tile([128, CS], F32, name="bden")
                    nc.vector.tensor_copy(out=bden, in_=pbd[:, :CS])
                    nc.vector.tensor_tensor(
                        out=attnT[:, hh, c * CS:(c + 1) * CS],
                        in0=po[:, :CS], in1=bden, op=mybir.AluOpType.mult)

                # wo + 0.125*h -> AR input (batched into one DMA per chunk)
                arbuf = arpool.tile([128, HT * CS], H16, name="arb")
                for g in range(HT // WOG):
                    wog = wopool.tile([128, WOG * DQ], H16, name="wog")
                    nc.sync.dma_start(
                        out=wog,
                        in_=_ap3(wo[l].ap()[g * WOG * 128:, :],
                                 128, WOG, DQ, DQ * 128))
                    for j in range(WOG):
                        m = g * WOG + j
                        pm = pspool.tile([128, 512], F32, name="mmps")
                        for kt in range(NH_C):
                            nc.tensor.matmul(
                                pm[:, :CS],
                                wog[:, j * DQ + kt * 128:
                                    j * DQ + (kt + 1) * 128],
                                attnT[:, kt, c * CS:(c + 1) * CS],
                                start=(kt == 0), stop=False)
                        nc.tensor.matmul(pm[:, :CS], id8,
                                         hsb[:, m * CS:(m + 1) * CS],
                                         start=False, stop=True)
                        nc.vector.tensor_copy(
                            out=arbuf[:, m * CS:(m + 1) * CS], in_=pm[:, :CS])
                store_ar_chunk(arbuf, ari[l][0][c][:, :])
                nc.gpsimd.collective_compute(
                    "AllReduce", mybir.AluOpType.add,
                    replica_groups=[list(range(NCORES))],
                    ins=[ari[l][0][c][:, :].opt()],
                    outs=[aro[l][0][c][:, :].opt()])

            # ---------------- phase B: FFN (chunk-outer) ----------------
            for c in range(CH):
                hsb = load_h_chunk(l, 1, c)
                st = stats(hsb, c)

                sT = sTpool.tile([128, FT * CS], H16, name="sT")
                for pair in range(FT):
                    wsl = wpool.tile([128, 2 * HT * 128], H16, name="wslab")
                    nc.sync.dma_start(
                        out=wsl,
                        in_=_ap3(wgu[l].ap()[(2 * pair) * 128:, :],
                                 128, 2, HT * 128, HT * 128 * 128))
                    pg = pspool.tile([128, 512], F32, name="mmps")
                    for kt in range(HT):
                        nc.tensor.matmul(
                            pg[:, :CS], wsl[:, kt * 128:(kt + 1) * 128],
                            hsb[:, kt * CS:(kt + 1) * CS],
                            start=(kt == 0), stop=(kt == HT - 1))
                    pu = pspool.tile([128, 512], F32, name="psc")
                    for kt in range(HT):
                        nc.tensor.matmul(
                            pu[:, :CS],
                            wsl[:, (HT + kt) * 128:(HT + kt + 1) * 128],
                            hsb[:, kt * CS:(kt + 1) * CS],
                            start=(kt == 0), stop=(kt == HT - 1))
                    gs = evpool.tile([128, CS], H16, name="gs")
                    nc.vector.tensor_tensor(out=gs, in0=pg[:, :CS],
                                            in1=st["binv"],
                                            op=mybir.AluOpType.mult)
                    us = evpool.tile([128, CS], H16, name="us")
                    nc.vector.tensor_tensor(out=us, in0=pu[:, :CS],
                                            in1=st["binv"],
                                            op=mybir.AluOpType.mult)
                    sg = evpool.tile([128, CS], H16, name="sg")
                    nc.scalar.activation(out=sg, in_=gs, func=AF.Sigmoid)
                    nc.vector.tensor_mul(out=sg, in0=sg, in1=gs)
                    nc.vector.tensor_tensor(
                        out=sT[:, pair * CS:(pair + 1) * CS],
                        in0=sg, in1=us, op=mybir.AluOpType.mult)

                arbuf = arpool.tile([128, HT * CS], H16, name="arb")
                for g in range(HT // WDG):
                    wdg = wdpool.tile([128, WDG * FT * 128], H16, name="wdg")
                    nc.sync.dma_start(
                        out=wdg,
                        in_=_ap3(wd[l].ap()[g * WDG * 128:, :],
                                 128, WDG, FT * 128, FT * 128 * 128))
                    for j in range(WDG):
                        m = g * WDG + j
                        pm = pspool.tile([128, 512], F32, name="mmps")
                        for kt in range(FT):
                            nc.tensor.matmul(
                                pm[:, :CS],
                                wdg[:, (j * FT + kt) * 128:
                                    (j * FT + kt + 1) * 128],
                                sT[:, kt * CS:(kt + 1) * CS],
                                start=(kt == 0), stop=False)
                        nc.tensor.matmul(pm[:, :CS], id8,
                                         hsb[:, m * CS:(m + 1) * CS],
                                         start=False, stop=True)
                        nc.vector.tensor_copy(
                            out=arbuf[:, m * CS:(m + 1) * CS], in_=pm[:, :CS])
                store_ar_chunk(arbuf, ari[l][1][c][:, :])
                nc.gpsimd.collective_compute(
                    "AllReduce", mybir.AluOpType.add,
                    replica_groups=[list(range(NCORES))],
                    ins=[ari[l][1][c][:, :].opt()],
                    outs=[aro[l][1][c][:, :].opt()])

        # ---------------- final norm + head ----------------
        wouts = wpool.tile([128, 2 * HT * 128], H16, name="wslab")
        nc.sync.dma_start(out=wouts[:, :HT * O_C], in_=wout_d.ap())
        bout_sb = cpool.tile([O_C, 1], F32, name="boutsb")
        nc.sync.dma_start(out=bout_sb, in_=bout_d.ap())
        for c in range(CH):
            hsb = load_h_chunk(L, 0, c)
            st = stats(hsb, c)
            pm = pspool.tile([128, 512], F32, name="mmps")
            for kt in range(HT):
                nc.tensor.matmul(
                    pm[0:O_C, :CS], wouts[:, kt * O_C:(kt + 1) * O_C],
                    hsb[:, kt * CS:(kt + 1) * CS],
                    start=(kt == 0), stop=(kt == HT - 1))
            ot = evpool.tile([O_C, CS], F32, name="otile")
            nc.vector.tensor_tensor(out=ot, in0=pm[0:O_C, :CS],
                                    in1=st["binv"][0:O_C, :],
                                    op=mybir.AluOpType.mult)
            nc.vector.tensor_scalar_add(out=ot, in0=ot, scalar1=bout_sb)
            nc.sync.dma_start(out=outT.ap()[:, c * CS:(c + 1) * CS], in_=ot)

    nc.compile()
    return nc


# ---------------- host side ----------------

def _rope_tables():
    inv = 1.0 / (ROPE_THETA ** (np.arange(0, HD, 2, dtype=np.float32) / HD))
    fr = np.arange(S, dtype=np.float32)[:, None] * inv[None, :]   # [S, 64]
    cos, sin = np.cos(fr), np.sin(fr)
    cos2 = np.concatenate([cos.T, cos.T], axis=0)                 # [128, S]
    sin2 = np.concatenate([sin.T, sin.T], axis=0)
    return (np.ascontiguousarray(cos2).astype(np.float16),
            np.ascontiguousarray(sin2).astype(np.float16))


def _pack_lhsT(w):
    """[K, M] natural -> [MT*128, KT*128] slab-major, partition-major."""
    K, M = w.shape
    KT, MT = K // 128, M // 128
    r = w.reshape(KT, 128, MT, 128).transpose(2, 1, 0, 3)
    return np.ascontiguousarray(r.reshape(MT * 128, KT * 128))


def _pack_rhs(w, n):
    """[K, N] natural -> [128, KT*N] (k-tile-major per partition)."""
    K = w.shape[0]
    KT = K // 128
    r = w.reshape(KT, 128, n).transpose(1, 0, 2)
    return np.ascontiguousarray(r.reshape(128, KT * n))


def _prep_in_maps(inputs):
    f32, f16 = np.float32, np.float16
    embed = np.asarray(inputs["embed"], f32)
    x = np.asarray(inputs["x"]).astype(np.int64).reshape(-1)
    h0T = np.ascontiguousarray(embed[x].T).astype(f16)            # [H, S]
    cos2, sin2 = _rope_tables()
    kk, jj = np.meshgrid(np.arange(128), np.arange(128), indexing="ij")
    maskT = np.where(kk <= jj, 0.0, NEG).astype(f16)              # [k, q]
    id8 = (np.eye(128) * 0.125).astype(f16)

    ln1 = np.asarray(inputs["ln1"], f32)
    ln2 = np.asarray(inputs["ln2"], f32)
    lnf = np.asarray(inputs["lnf"], f32)
    Wq = np.asarray(inputs["Wq"], f32)
    Wk = np.asarray(inputs["Wk"], f32)
    Wv = np.asarray(inputs["Wv"], f32)
    Wo = np.asarray(inputs["Wo"], f32)
    Wg = np.asarray(inputs["Wg"], f32)
    Wu = np.asarray(inputs["Wu"], f32)
    Wd = np.asarray(inputs["Wd"], f32)
    Wout = np.asarray(inputs["Wout"], f32) * lnf[:, None]
    bout = np.asarray(inputs["bout"], f32)

    in_maps = []
    for c in range(NCORES):
        m = {"h0T": h0T, "cos2": cos2, "sin2": sin2, "maskT": maskT,
             "id8": id8}
        csl = slice(c * DQ, (c + 1) * DQ)
        fsl = slice(c * F_C, (c + 1) * F_C)
        for l in range(L):
            wq = Wq[l] * ln1[l][:, None] / np.sqrt(HD)
            wk = Wk[l] * ln1[l][:, None]
            wvn = Wv[l] * ln1[l][:, None]
            wg = Wg[l] * ln2[l][:, None]
            wu = Wu[l] * ln2[l][:, None]
            qk = np.concatenate([wq[:, csl], wk[:, csl]], axis=1)  # [H, 2DQ]
            m[f"wqk{l}"] = _pack_lhsT(qk.astype(f16))
            m[f"wv{l}"] = _pack_rhs(np.ascontiguousarray(
                wvn[:, csl]).astype(f16), DQ)
            m[f"wo{l}"] = _pack_lhsT(np.ascontiguousarray(
                Wo[l][csl, :]).astype(f16))
            gu = np.zeros((H, GUM * 128), f32)
            gc = np.zeros((H, F_CP), f32)
            uc = np.zeros((H, F_CP), f32)
            gc[:, :F_C] = wg[:, fsl]
            uc[:, :F_C] = wu[:, fsl]
            for t in range(FT):
                gu[:, (2 * t) * 128:(2 * t + 1) * 128] = \
                    gc[:, t * 128:(t + 1) * 128]
                gu[:, (2 * t + 1) * 128:(2 * t + 2) * 128] = \
                    uc[:, t * 128:(t + 1) * 128]
            m[f"wgu{l}"] = _pack_lhsT(gu.astype(f16))
            wd_c = np.zeros((F_CP, H), f32)
            wd_c[:F_C, :] = Wd[l][fsl, :]
            m[f"wd{l}"] = _pack_lhsT(wd_c.astype(f16))
        osl = slice(c * O_C, (c + 1) * O_C)
        m["wout"] = _pack_rhs(np.ascontiguousarray(Wout[:, osl]).astype(f16),
                              O_C)
        m["bout"] = np.ascontiguousarray(bout[osl][:, None]).astype(f32)
        in_maps.append(m)
    return in_maps


_NC = None


def _get_nc():
    global _NC
    if _NC is None:
        _NC = build_nc()
    return _NC


def kernel(**inputs):
    nc = _get_nc()
    in_maps = _prep_in_maps(inputs)
    res = run_bass_kernel_spmd(nc, in_maps, core_ids=list(range(NCORES)))
    out = np.empty((B, S, O), np.float32)
    for c in range(NCORES):
        out[0, :, c * O_C:(c + 1) * O_C] = res.results[c]["outT"].T
    return out



# revision 61
# speedup vs baseline: 19.5226x; 19.5226x over previous
"""Trainium2 Bass kernel v3: 2-layer LLaMA-style transformer, 8-way tensor
parallel, sequence-chunked pipeline.

v3 changes vs v2:
- CH=2 chunks of CS=512 (was 4x256): halves weight HBM traffic (weights
  are re-streamed per chunk) and halves matmul count (N=512 free dims),
  keeping the PE's HAM clock warm.
- Single shared PSUM "acc" tag (4 banks) + po/row/bc tags (3 banks).
- AR input staged in ARG-slab groups -> 1MB DMAs, no full-chunk arbuf.
- g/u slabs loaded as two separate 8KB-tag DMAs (smaller SBUF slots).

Per core:
- Activations feature-major [feat, seq] in SBUF; weights host-packed into
  partition-major slabs so every weight DMA is one long-line transfer.
- Sequence processed in CH chunks; each half-layer's AllReduce issued per
  chunk so collectives overlap neighboring chunks' compute.
- RMSNorm 1/rms folded into PSUM evictions (and into rope tables for q/k);
  ln weights folded into weights on host.
- Residual folded into the AllReduce: AR input = partial_out + 0.125*h via
  an extra identity matmul into the same PSUM accumulation, so
  h_new = AllReduce(.) with no separate residual pass.
"""

from contextlib import ExitStack

import numpy as np

import concourse.bass as bass
import concourse.bacc as bacc
import concourse.tile as tile
from concourse import mybir
from concourse.bass_utils import run_bass_kernel_spmd

F32 = mybir.dt.float32
H16 = mybir.dt.float16
AF = mybir.ActivationFunctionType

B, S = 1, 1024
V, H, NH, HD, F, L, O = 32000, 4096, 32, 128, 11008, 2, 512
NCORES = 8
NH_C = NH // NCORES      # 4 heads/core
DQ = NH_C * HD           # 512
F_C = F // NCORES        # 1376
F_CP = 1408              # 11*128
O_C = O // NCORES        # 64
HT = H // 128            # 32
FT = F_CP // 128         # 11
ROPE_THETA = 10000.0
EPS = 1e-6
NEG = -30000.0

CH = 2                   # sequence chunks
CS = S // CH             # 512
CT = CS // 128           # 4 s-tiles per chunk

QKM = 2 * DQ // 128      # 8 qk slabs
GUM = 2 * FT             # 22 gu slabs (g0,u0,g1,u1,...)
WVG = 4                  # wv k-tiles per DMA group
WOG = 4                  # wo slabs per DMA group
WDG = 1                  # wd slabs per DMA group
ARG = 2                  # out-slabs per AR staging group


def _ap3(ap2, p, d1, d2, s1):
    """3D view [(p,128), (d1, stride s1), (d2, contiguous)] of a 2D AP."""
    return bass.AP(tensor=ap2.tensor, offset=ap2.offset,
                   ap=[[ap2.ap[0][0], p], [s1, d1], [ap2.ap[1][0], d2]])


def build_nc():
    nc = bacc.Bacc("TRN2", target_bir_lowering=False, debug=False,
                   num_devices=NCORES)

    # ---- kernel I/O ----
    h0T = nc.dram_tensor("h0T", [H, S], H16, kind="ExternalInput")
    cos2_d = nc.dram_tensor("cos2", [128, S], H16, kind="ExternalInput")
    sin2_d = nc.dram_tensor("sin2", [128, S], H16, kind="ExternalInput")
    maskT_d = nc.dram_tensor("maskT", [128, 128], H16, kind="ExternalInput")
    wqk = [nc.dram_tensor(f"wqk{l}", [QKM * 128, HT * 128], H16,
                          kind="ExternalInput") for l in range(L)]
    wv = [nc.dram_tensor(f"wv{l}", [128, HT * DQ], H16,
                         kind="ExternalInput") for l in range(L)]
    wo = [nc.dram_tensor(f"wo{l}", [HT * 128, DQ], H16,
                         kind="ExternalInput") for l in range(L)]
    wgu = [nc.dram_tensor(f"wgu{l}", [GUM * 128, HT * 128], H16,
                          kind="ExternalInput") for l in range(L)]
    wd = [nc.dram_tensor(f"wd{l}", [HT * 128, FT * 128], H16,
                         kind="ExternalInput") for l in range(L)]
    wout_d = nc.dram_tensor("wout", [128, HT * O_C], H16,
                            kind="ExternalInput")
    bout_d = nc.dram_tensor("bout", [O_C, 1], F32, kind="ExternalInput")
    outT = nc.dram_tensor("outT", [O_C, S], F32, kind="ExternalOutput")

    with tile.TileContext(nc) as tc, ExitStack() as ctx:
        ec = ctx.enter_context
        dpool = ec(tc.tile_pool(name="drams", bufs=1, space="DRAM"))
        cpool = ec(tc.tile_pool(name="consts", bufs=1))
        apool = ec(tc.tile_pool(name="acts", bufs=1))      # qkr/vnat
        atpool = ec(tc.tile_pool(name="attnT", bufs=2))
        hpool = ec(tc.tile_pool(name="hsb", bufs=2))       # h chunks
        stpool = ec(tc.tile_pool(name="stp", bufs=2))      # binv/cosI/...
        sqpool = ec(tc.tile_pool(name="sqp", bufs=4))      # squares
        sTpool = ec(tc.tile_pool(name="sTs", bufs=1))      # swiglu out
        wpool = ec(tc.tile_pool(name="wslab", bufs=3))     # qk/g/u slabs 8KB
        wvpool = ec(tc.tile_pool(name="wvg", bufs=2))      # wv groups
        wopool = ec(tc.tile_pool(name="wog", bufs=2))      # wo groups
        wdpool = ec(tc.tile_pool(name="wdg", bufs=2))      # wd groups
        spool = ec(tc.tile_pool(name="small", bufs=2))
        aspool = ec(tc.tile_pool(name="asb", bufs=1))      # attn exp tiles
        evpool = ec(tc.tile_pool(name="evict", bufs=2))
        arpool = ec(tc.tile_pool(name="arb", bufs=2))      # AR-input stage
        accpool = ec(tc.tile_pool(name="acc", bufs=4, space="PSUM"))
        popool = ec(tc.tile_pool(name="pso", bufs=1, space="PSUM"))
        rowpool = ec(tc.tile_pool(name="psr", bufs=1, space="PSUM"))
        bcpool = ec(tc.tile_pool(name="psb", bufs=1, space="PSUM"))

        # ---- constants ----
        cos2 = cpool.tile([128, S], H16, name="cos2sb")
        nc.sync.dma_start(out=cos2, in_=cos2_d.ap())
        sin2 = cpool.tile([128, S], H16, name="sin2sb")
        nc.sync.dma_start(out=sin2, in_=sin2_d.ap())
        maskT = cpool.tile([128, 128], H16, name="maskTsb")
        nc.sync.dma_start(out=maskT, in_=maskT_d.ap())
        maskF = cpool.tile([128, (CT - 1) * 128], H16, name="maskFsb")
        nc.vector.memset(maskF, NEG)
        ones128 = cpool.tile([128, 1], H16, name="ones128")
        nc.vector.memset(ones128, 1.0)
        ones1 = cpool.tile([1, 128], F32, name="ones1")
        nc.vector.memset(ones1, 1.0)
        eps1 = cpool.tile([1, 1], F32, name="eps1")
        nc.vector.memset(eps1, EPS)
        bout_sb = cpool.tile([O_C, 1], F32, name="boutsb")
        nc.sync.dma_start(out=bout_sb, in_=bout_d.ap())

        # ---- persistent per-layer activations ----
        qkr = apool.tile([128, 2 * NH_C, S], H16, name="qkr")     # 16KB/part
        vnat = apool.tile([128, S // 128, DQ], H16, name="vnat")  # 8KB/part

        # ---- DRAM scratch ----
        invd = dpool.tile([1, S], F32, name="invd")
        # AR input/output split in 4 row-quarters of [H/4, CS] so each
        # collective fires as soon as its 8 out-slabs are staged; bursts
        # spread over the producing section and only the last quarter's
        # latency is exposed at phase boundaries.
        HQ = H // 4
        ari = [[[[dpool.tile([HQ, CS], H16, name=f"ari{l}_{hf}_{c}_{q}")
                  for q in range(4)]
                 for c in range(CH)] for hf in range(2)] for l in range(L)]
        aro = [[[[dpool.tile([HQ, CS], H16, name=f"aro{l}_{hf}_{c}_{q}",
                             addr_space="Shared")
                  for q in range(4)]
                 for c in range(CH)] for hf in range(2)] for l in range(L)]

        def h_src_ap_q(l, hf, c, q):
            if l == 0 and hf == 0:
                return h0T.ap()[q * HQ:(q + 1) * HQ, c * CS:(c + 1) * CS]
            if hf == 0:
                return aro[l - 1][1][c][q][:, :]
            return aro[l][0][c][q][:, :]

        htiles = {}

        def emit_h_load(l, hf, c):
            """[H, CS] dram slice -> SBUF [128, HT*CS] (kt-major), split in
            quarter DMAs so consumers can start on early k-tiles. Emitted
            right after the producing AllReduce so the transfer overlaps the
            previous phase's compute."""
            t = hpool.tile([128, HT * CS], H16, name="hsb")
            kq = HT // 4
            for q in range(4):
                src = h_src_ap_q(l, hf, c, q)
                nc.sync.dma_start(
                    out=_ap3(t[:, q * kq * CS:(q + 1) * kq * CS],
                             128, kq, CS, CS),
                    in_=_ap3(src, 128, kq, CS, src.ap[0][0] * 128))
            htiles[(l, hf, c)] = t

        def stats(hsb, c, want_invcol=False, want_rope=False):
            """ssq -> 1/rms; binv16 [128,CS] broadcast; optional invcol
            [128,CT] and rope tables pre-scaled by inv."""
            pss = rowpool.tile([1, 512], F32, name="psrow")
            for kt in range(HT):
                sqt = sqpool.tile([128, CS], H16, name="sqt")
                sl = hsb[:, kt * CS:(kt + 1) * CS]
                nc.vector.tensor_mul(out=sqt, in0=sl, in1=sl)
                nc.tensor.matmul(pss[:, :CS], ones128, sqt,
                                 start=(kt == 0), stop=(kt == HT - 1))
            srow = spool.tile([1, CS], F32, name="srow")
            nc.scalar.activation(out=srow, in_=pss[:, :CS], func=AF.Sqrt,
                                 bias=eps1, scale=1.0 / H)
            inv = spool.tile([1, CS], F32, name="invrow")
            nc.vector.reciprocal(out=inv, in_=srow)
            pbc = bcpool.tile([128, 512], F32, name="psbc")
            nc.tensor.matmul(pbc[:, :CS], ones1, inv, start=True, stop=True)
            binv = stpool.tile([128, CS], H16, name="binv")
            nc.vector.tensor_copy(out=binv, in_=pbc[:, :CS])
            out = {"binv": binv}
            if want_invcol:
                dsl = invd[:, c * CS:(c + 1) * CS]
                nc.sync.dma_start(out=dsl, in_=inv)
                invcol = stpool.tile([128, CT], F32, name="invcol")
                nc.sync.dma_start(out=invcol, in_=bass.AP(
                    tensor=dsl.tensor, offset=dsl.offset,
                    ap=[[1, 128], [128, CT]]))
                out["invcol"] = invcol
            if want_rope:
                csl = slice(c * CS, (c + 1) * CS)
                cosI = stpool.tile([128, CS], H16, name="cosI")
                nc.vector.tensor_tensor(out=cosI, in0=cos2[:, csl], in1=binv,
                                        op=mybir.AluOpType.mult)
                sinI = stpool.tile([128, CS], H16, name="sinI")
                nc.vector.tensor_tensor(out=sinI, in0=sin2[:, csl], in1=binv,
                                        op=mybir.AluOpType.mult)
                out["cosI"] = cosI
                out["sinI"] = sinI
            return out

        GPQ = HQ // (ARG * 128)     # stage-groups per AR quarter

        def ar_stage_store(l, hf, c, g2, arst):
            """One DMA: SBUF [128, ARG*CS] -> rows of an AR quarter; when
            the quarter is complete, fire its AllReduce."""
            q, g = g2 // GPQ, g2 % GPQ
            dst = ari[l][hf][c][q][g * ARG * 128:(g + 1) * ARG * 128, :]
            nc.sync.dma_start(
                out=_ap3(dst, 128, ARG, CS, dst.ap[0][0] * 128),
                in_=_ap3(arst[:, :], 128, ARG, CS, CS))
            if g == GPQ - 1:
                nc.gpsimd.collective_compute(
                    "AllReduce", mybir.AluOpType.add,
                    replica_groups=[list(range(NCORES))],
                    ins=[ari[l][hf][c][q][:, :].opt()],
                    outs=[aro[l][hf][c][q][:, :].opt()])

        # =========================== layers ===========================
        for c in range(CH):
            emit_h_load(0, 0, c)
        for l in range(L):
            # ---------------- phase A: attention --------------------
            # Order: stats+qk+v for BOTH chunks first (DMA-heavy), then
            # attention+wo per chunk. The attention sections are DMA-free,
            # so each chunk's AllReduce burst overlaps them instead of
            # starving the next chunk's weight streams. qk slabs are
            # loaded once and applied to both chunks.
            hsbs = [htiles[(l, 0, c)] for c in range(CH)]
            sts = [stats(hsbs[c], c, want_invcol=True, want_rope=True)
                   for c in range(CH)]

            # qk projections + fused rope (slab-outer, chunks inner)
            for m in range(QKM):
                wsl = wpool.tile([128, HT * 128], H16, name="wslab")
                nc.sync.dma_start(
                    out=wsl, in_=wqk[l].ap()[m * 128:(m + 1) * 128, :])
                for c in range(CH):
                    st = sts[c]
                    pm = accpool.tile([128, 512], F32, name="psacc")
                    for kt in range(HT):
                        nc.tensor.matmul(
                            pm[:, :CS], wsl[:, kt * 128:(kt + 1) * 128],
                            hsbs[c][:, kt * CS:(kt + 1) * CS],
                            start=(kt == 0), stop=(kt == HT - 1))
                    # rope fused into eviction (tables carry 1/rms);
                    # temps read the PSUM tile directly
                    rot = evpool.tile([128, CS], H16, name="rot")
                    nc.scalar.mul(out=rot[0:64, :], in_=pm[64:128, :CS],
                                  mul=-1.0)
                    nc.scalar.copy(out=rot[64:128, :], in_=pm[0:64, :CS])
                    t1 = evpool.tile([128, CS], H16, name="ropet1")
                    nc.vector.tensor_tensor(out=t1, in0=pm[:, :CS],
                                            in1=st["cosI"],
                                            op=mybir.AluOpType.mult)
                    t2 = evpool.tile([128, CS], H16, name="ropet2")
                    nc.vector.tensor_tensor(out=t2, in0=rot, in1=st["sinI"],
                                            op=mybir.AluOpType.mult)
                    nc.vector.tensor_add(
                        out=qkr[:, m, c * CS:(c + 1) * CS], in0=t1, in1=t2)

            # v projection (natural layout), wv streamed per chunk
            for c in range(CH):
                hsb, st = hsbs[c], sts[c]
                pvs = [accpool.tile([128, 512], F32, name="psacc")
                       for _ in range(CT)]
                for g in range(HT // WVG):
                    wvt = wvpool.tile([128, WVG * DQ], H16, name="wvg")
                    nc.sync.dma_start(
                        out=wvt,
                        in_=wv[l].ap()[:, g * WVG * DQ:(g + 1) * WVG * DQ])
                    for j in range(WVG):
                        kt = g * WVG + j
                        for stt in range(CT):
                            nc.tensor.matmul(
                                pvs[stt],
                                hsb[:, kt * CS + stt * 128:
                                    kt * CS + (stt + 1) * 128],
                                wvt[:, j * DQ:(j + 1) * DQ],
                                start=(kt == 0), stop=(kt == HT - 1))
                for stt in range(CT):
                    nc.vector.tensor_scalar_mul(
                        out=vnat[:, c * CT + stt, :], in0=pvs[stt],
                        scalar1=st["invcol"][:, stt:stt + 1])

            # attention + wo + AR, per chunk
            for c in range(CH):
                hsb, st = htiles.pop((l, 0, c)), sts[c]
                nkt = CT * (c + 1)
                attnT = atpool.tile([128, NH_C, CS], H16, name="attnT")
                for hh in range(NH_C):
                    a_sb = aspool.tile([128, S // 128, CS], H16, name="asb")
                    for kt in range(nkt):
                        psc = accpool.tile([128, 512], F32, name="psacc")
                        nc.tensor.matmul(
                            psc[:, :CS],
                            qkr[:, NH_C + hh, kt * 128:(kt + 1) * 128],
                            qkr[:, hh, c * CS:(c + 1) * CS],
                            start=True, stop=True)
                        dj = kt - CT * c
                        if dj >= 1:
                            nc.vector.tensor_add(
                                out=psc[:, 0:dj * 128], in0=psc[:, 0:dj * 128],
                                in1=maskF[:, 0:dj * 128])
                        if 0 <= dj < CT:
                            nc.vector.tensor_add(
                                out=psc[:, dj * 128:(dj + 1) * 128],
                                in0=psc[:, dj * 128:(dj + 1) * 128],
                                in1=maskT)
                        nc.scalar.activation(out=a_sb[:, kt, :],
                                             in_=psc[:, :CS], func=AF.Exp)
                    po = popool.tile([128, 512], F32, name="pso")
                    pd = rowpool.tile([1, 512], F32, name="psrow")
                    for kt in range(nkt):
                        nc.tensor.matmul(po[:, :CS],
                                         vnat[:, kt, hh * 128:(hh + 1) * 128],
                                         a_sb[:, kt, :],
                                         start=(kt == 0), stop=(kt == nkt - 1))
                        nc.tensor.matmul(pd[:, :CS], ones128, a_sb[:, kt, :],
                                         start=(kt == 0), stop=(kt == nkt - 1))
                    den = spool.tile([1, CS], F32, name="srow")
                    nc.vector.reciprocal(out=den, in_=pd[:, :CS])
                    pbd = bcpool.tile([128, 512], F32, name="psbc")
                    nc.tensor.matmul(pbd[:, :CS], ones1, den,
                                     start=True, stop=True)
                    bden = evpool.tile([128, CS], H16, name="bden")
                    nc.vector.tensor_copy(out=bden, in_=pbd[:, :CS])
                    nc.vector.tensor_tensor(
                        out=attnT[:, hh, :],
                        in0=po[:, :CS], in1=bden, op=mybir.AluOpType.mult)

                # wo + 0.125*h -> AR input (staged in ARG-slab groups)
                for g in range(HT // WOG):
                    wog = wopool.tile([128, WOG * DQ], H16, name="wog")
                    nc.sync.dma_start(
                        out=wog,
                        in_=_ap3(wo[l].ap()[g * WOG * 128:, :],
                                 128, WOG, DQ, DQ * 128))
                    for j in range(WOG):
                        m = g * WOG + j
                        if m % ARG == 0:
                            arst = arpool.tile([128, ARG * CS], H16,
                                               name="arst")
                        pm = accpool.tile([128, 512], F32, name="psacc")
                        for kt in range(NH_C):
                            nc.tensor.matmul(
                                pm[:, :CS],
                                wog[:, j * DQ + kt * 128:
                                    j * DQ + (kt + 1) * 128],
                                attnT[:, kt, :],
                                start=(kt == 0), stop=(kt == NH_C - 1))
                        # AR input = partial_out + h/8 in one fused DVE op
                        nc.vector.scalar_tensor_tensor(
                            out=arst[:, (m % ARG) * CS:(m % ARG + 1) * CS],
                            in0=hsb[:, m * CS:(m + 1) * CS], scalar=0.125,
                            in1=pm[:, :CS], op0=mybir.AluOpType.mult,
                            op1=mybir.AluOpType.add)
                        if m % ARG == ARG - 1:
                            ar_stage_store(l, 0, c, m // ARG, arst)
                emit_h_load(l, 1, c)

            # ---------------- phase B: FFN (chunk-outer) ----------------
            for c in range(CH):
                hsb = htiles.pop((l, 1, c))
                st = stats(hsb, c)

                sT = sTpool.tile([128, FT * CS], H16, name="sT")
                for pair in range(FT):
                    wsg = wpool.tile([128, HT * 128], H16, name="wslab")
                    nc.sync.dma_start(
                        out=wsg, in_=wgu[l].ap()[
                            (2 * pair) * 128:(2 * pair + 1) * 128, :])
                    wsu = wpool.tile([128, HT * 128], H16, name="wslab")
                    nc.sync.dma_start(
                        out=wsu, in_=wgu[l].ap()[
                            (2 * pair + 1) * 128:(2 * pair + 2) * 128, :])
                    pg = accpool.tile([128, 512], F32, name="psacc")
                    for kt in range(HT):
                        nc.tensor.matmul(
                            pg[:, :CS], wsg[:, kt * 128:(kt + 1) * 128],
                            hsb[:, kt * CS:(kt + 1) * CS],
                            start=(kt == 0), stop=(kt == HT - 1))
                    pu = accpool.tile([128, 512], F32, name="psacc")
                    for kt in range(HT):
                        nc.tensor.matmul(
                            pu[:, :CS], wsu[:, kt * 128:(kt + 1) * 128],
                            hsb[:, kt * CS:(kt + 1) * CS],
                            start=(kt == 0), stop=(kt == HT - 1))
                    gs = evpool.tile([128, CS], H16, name="gs")
                    nc.vector.tensor_tensor(out=gs, in0=pg[:, :CS],
                                            in1=st["binv"],
                                            op=mybir.AluOpType.mult)
                    us = evpool.tile([128, CS], H16, name="us")
                    nc.vector.tensor_tensor(out=us, in0=pu[:, :CS],
                                            in1=st["binv"],
                                            op=mybir.AluOpType.mult)
                    sg = evpool.tile([128, CS], H16, name="sg")
                    nc.scalar.activation(out=sg, in_=gs, func=AF.Sigmoid)
                    nc.vector.tensor_mul(out=sg, in0=sg, in1=gs)
                    nc.vector.tensor_tensor(
                        out=sT[:, pair * CS:(pair + 1) * CS],
                        in0=sg, in1=us, op=mybir.AluOpType.mult)

                for g in range(HT // WDG):
                    wdg = wdpool.tile([128, WDG * FT * 128], H16, name="wdg")
                    nc.sync.dma_start(
                        out=wdg,
                        in_=_ap3(wd[l].ap()[g * WDG * 128:, :],
                                 128, WDG, FT * 128, FT * 128 * 128))
                    for j in range(WDG):
                        m = g * WDG + j
                        if m % ARG == 0:
                            arst = arpool.tile([128, ARG * CS], H16,
                                               name="arst")
                        pm = accpool.tile([128, 512], F32, name="psacc")
                        for kt in range(FT):
                            nc.tensor.matmul(
                                pm[:, :CS],
                                wdg[:, (j * FT + kt) * 128:
                                    (j * FT + kt + 1) * 128],
                                sT[:, kt * CS:(kt + 1) * CS],
                                start=(kt == 0), stop=(kt == FT - 1))
                        nc.vector.scalar_tensor_tensor(
                            out=arst[:, (m % ARG) * CS:(m % ARG + 1) * CS],
                            in0=hsb[:, m * CS:(m + 1) * CS], scalar=0.125,
                            in1=pm[:, :CS], op0=mybir.AluOpType.mult,
                            op1=mybir.AluOpType.add)
                        if m % ARG == ARG - 1:
                            ar_stage_store(l, 1, c, m // ARG, arst)
                emit_h_load(l + 1, 0, c) if l + 1 < L else emit_h_load(L, 0, c)

        # ---------------- final norm + head ----------------
        wouts = wpool.tile([128, HT * O_C], H16, name="wslab")
        nc.sync.dma_start(out=wouts, in_=wout_d.ap())
        for c in range(CH):
            hsb = htiles.pop((L, 0, c))
            st = stats(hsb, c)
            pm = accpool.tile([128, 512], F32, name="psacc")
            for kt in range(HT):
                nc.tensor.matmul(
                    pm[0:O_C, :CS], wouts[:, kt * O_C:(kt + 1) * O_C],
                    hsb[:, kt * CS:(kt + 1) * CS],
                    start=(kt == 0), stop=(kt == HT - 1))
            ot = evpool.tile([O_C, CS], F32, name="otile")
            nc.vector.tensor_tensor(out=ot, in0=pm[0:O_C, :CS],
                                    in1=st["binv"][0:O_C, :],
                                    op=mybir.AluOpType.mult)
            nc.vector.tensor_scalar_add(out=ot, in0=ot, scalar1=bout_sb)
            nc.sync.dma_start(out=outT.ap()[:, c * CS:(c + 1) * CS], in_=ot)

    nc.compile()
    return nc


# ---------------- host side ----------------

def _rope_tables():
    inv = 1.0 / (ROPE_THETA ** (np.arange(0, HD, 2, dtype=np.float32) / HD))
    fr = np.arange(S, dtype=np.float32)[:, None] * inv[None, :]   # [S, 64]
    cos, sin = np.cos(fr), np.sin(fr)
    cos2 = np.concatenate([cos.T, cos.T], axis=0)                 # [128, S]
    sin2 = np.concatenate([sin.T, sin.T], axis=0)
    return (np.ascontiguousarray(cos2).astype(np.float16),
            np.ascontiguousarray(sin2).astype(np.float16))


def _pack_lhsT(w):
    """[K, M] natural -> [MT*128, KT*128] slab-major, partition-major."""
    K, M = w.shape
    KT, MT = K // 128, M // 128
    r = w.reshape(KT, 128, MT, 128).transpose(2, 1, 0, 3)
    return np.ascontiguousarray(r.reshape(MT * 128, KT * 128))


def _pack_rhs(w, n):
    """[K, N] natural -> [128, KT*N] (k-tile-major per partition)."""
    K = w.shape[0]
    KT = K // 128
    r = w.reshape(KT, 128, n).transpose(1, 0, 2)
    return np.ascontiguousarray(r.reshape(128, KT * n))


def _prep_in_maps(inputs):
    f32, f16 = np.float32, np.float16
    embed = np.asarray(inputs["embed"], f32)
    x = np.asarray(inputs["x"]).astype(np.int64).reshape(-1)
    h0T = np.ascontiguousarray(embed[x].T).astype(f16)            # [H, S]
    cos2, sin2 = _rope_tables()
    kk, jj = np.meshgrid(np.arange(128), np.arange(128), indexing="ij")
    maskT = np.where(kk <= jj, 0.0, NEG).astype(f16)              # [k, q]

    ln1 = np.asarray(inputs["ln1"], f32)
    ln2 = np.asarray(inputs["ln2"], f32)
    lnf = np.asarray(inputs["lnf"], f32)
    Wq = np.asarray(inputs["Wq"], f32)
    Wk = np.asarray(inputs["Wk"], f32)
    Wv = np.asarray(inputs["Wv"], f32)
    Wo = np.asarray(inputs["Wo"], f32)
    Wg = np.asarray(inputs["Wg"], f32)
    Wu = np.asarray(inputs["Wu"], f32)
    Wd = np.asarray(inputs["Wd"], f32)
    Wout = np.asarray(inputs["Wout"], f32) * lnf[:, None]
    bout = np.asarray(inputs["bout"], f32)

    in_maps = []
    for c in range(NCORES):
        m = {"h0T": h0T, "cos2": cos2, "sin2": sin2, "maskT": maskT}
        csl = slice(c * DQ, (c + 1) * DQ)
        fsl = slice(c * F_C, (c + 1) * F_C)
        for l in range(L):
            wq = Wq[l] * ln1[l][:, None] / np.sqrt(HD)
            wk = Wk[l] * ln1[l][:, None]
            wvn = Wv[l] * ln1[l][:, None]
            wg = Wg[l] * ln2[l][:, None]
            wu = Wu[l] * ln2[l][:, None]
            qk = np.concatenate([wq[:, csl], wk[:, csl]], axis=1)  # [H, 2DQ]
            m[f"wqk{l}"] = _pack_lhsT(qk.astype(f16))
            m[f"wv{l}"] = _pack_rhs(np.ascontiguousarray(
                wvn[:, csl]).astype(f16), DQ)
            m[f"wo{l}"] = _pack_lhsT(np.ascontiguousarray(
                Wo[l][csl, :]).astype(f16))
            gu = np.zeros((H, GUM * 128), f32)
            gc = np.zeros((H, F_CP), f32)
            uc = np.zeros((H, F_CP), f32)
            gc[:, :F_C] = wg[:, fsl]
            uc[:, :F_C] = wu[:, fsl]
            for t in range(FT):
                gu[:, (2 * t) * 128:(2 * t + 1) * 128] = \
                    gc[:, t * 128:(t + 1) * 128]
                gu[:, (2 * t + 1) * 128:(2 * t + 2) * 128] = \
                    uc[:, t * 128:(t + 1) * 128]
            m[f"wgu{l}"] = _pack_lhsT(gu.astype(f16))
            wd_c = np.zeros((F_CP, H), f32)
            wd_c[:F_C, :] = Wd[l][fsl, :]
            m[f"wd{l}"] = _pack_lhsT(wd_c.astype(f16))
        osl = slice(c * O_C, (c + 1) * O_C)
        m["wout"] = _pack_rhs(np.ascontiguousarray(Wout[:, osl]).astype(f16),
                              O_C)
        m["bout"] = np.ascontiguousarray(bout[osl][:, None]).astype(f32)
        in_maps.append(m)
    return in_maps


_NC = None


def _get_nc():
    global _NC
    if _NC is None:
        _NC = build_nc()
    return _NC


def kernel(**inputs):
    nc = _get_nc()
    in_maps = _prep_in_maps(inputs)
    res = run_bass_kernel_spmd(nc, in_maps, core_ids=list(range(NCORES)))
    out = np.empty((B, S, O), np.float32)
    for c in range(NCORES):
        out[0, :, c * O_C:(c + 1) * O_C] = res.results[c]["outT"].T
    return out
